# revision 1
# baseline (speedup 1.0000x reference)
"""BottleneckLSTMCell fused kernel for 8 Trainium2 NeuronCores.

Sharding: data-parallel over batch (B=8 -> 1 image per core). Each core runs
the full cell for its image:

  phase A: xw = dw3x3(x) (+bias folded into the Wy bias); i = Wy @ [h; xw] + b
  phase B: b = dw3x3(i); four 1x1 gate matmuls; LSTM pointwise -> (ch, cc)

Depthwise convs run on the tensor engine as 9 per-tap diagonal matmuls
accumulating in PSUM, reading from zero-padded SBUF-resident images so SAME
padding comes free (x arrives zero-padded from the host; the padded i image
borders are painted once via the scalar engine). All matmuls use float32r
(full-rate fp32 streaming on TRN2).
"""

import sys

if '/opt/trn_rl_repo' not in sys.path:
    sys.path.insert(0, '/opt/trn_rl_repo')

import numpy as np

import concourse.bass as bass  # noqa: F401
from concourse import bacc
import concourse.mybir as mybir
from concourse.tile import TileContext
from concourse.bass_utils import run_bass_kernel_spmd

F32 = mybir.dt.float32
F32R = mybir.dt.float32r
AF = mybir.ActivationFunctionType
ALU = mybir.AluOpType

B, CIN, CH, HW = 8, 320, 512, 64
PIX = HW * HW          # 4096
NCORES = 8
NCHUNK = 8             # spatial slabs of 8 rows (512 px)
XCH = [128, 128, 64]   # x channel chunk sizes (320)


def build_nc():
    nc = bacc.Bacc(None, target_bir_lowering=False, num_devices=NCORES)

    xd = nc.dram_tensor("x", (CIN, 66, 66), F32R, kind="ExternalInput")
    hd = nc.dram_tensor("h", (CH, PIX), F32R, kind="ExternalInput")
    cd = nc.dram_tensor("c", (CH, PIX), F32, kind="ExternalInput")
    wyd = nc.dram_tensor("wy", (128, 7, 512), F32R, kind="ExternalInput")
    wybd = nc.dram_tensor("wyb", (128, 4), F32, kind="ExternalInput")
    wgd = nc.dram_tensor("wg", (128, 16, 512), F32R, kind="ExternalInput")
    dwxd = nc.dram_tensor("dwx", (128, 3, 1152), F32R, kind="ExternalInput")
    dwid = nc.dram_tensor("dwi", (128, 4, 1152), F32R, kind="ExternalInput")
    zd = nc.dram_tensor("zz", (128, 128), F32R, kind="ExternalInput")
    ccd = nc.dram_tensor("occ", (CH, PIX), F32, kind="ExternalOutput")
    chd = nc.dram_tensor("och", (CH, PIX), F32, kind="ExternalOutput")

    x_ap, h_ap, c_ap = xd.ap(), hd.ap(), cd.ap()
    cc_ap, ch_ap = ccd.ap(), chd.ap()

    taps = [(t // 3 - 1, t % 3 - 1) for t in range(9)]

    with TileContext(nc) as tc:
        with tc.tile_pool(name="persist", bufs=1) as pp, \
             tc.tile_pool(name="wB", bufs=1) as wB:
            # small zeros tile used to paint halo borders via ACT
            zt = pp.tile([128, 66], F32R, tag="zt", name="zt")
            nc.sync.dma_start(out=zt[:], in_=zd.ap()[:, 0:66])

            # i image, zero-padded on all sides: [66 rows x 66 cols]
            i_pad = [pp.tile([128, 66, 66], F32R, tag=f"ipad{m}", name=f"ipad{m}")
                     for m in range(4)]
            for m in range(4):
                nc.scalar.copy(i_pad[m][:, 0, :], zt[:, :])
                nc.scalar.copy(i_pad[m][:, 65, :], zt[:, :])
                nc.scalar.copy(i_pad[m][:, :, 0], zt[:, :])
                nc.scalar.copy(i_pad[m][:, :, 65], zt[:, :])

            # ---------------- phase A ----------------
            with (
                tc.tile_pool(name="wA", bufs=1) as wA,
                tc.tile_pool(name="sA", bufs=2) as sA,
                tc.tile_pool(name="psxw", bufs=3, space="PSUM") as psxw,
                tc.tile_pool(name="psi", bufs=4, space="PSUM") as psi,
            ):
                def emit_slab_inputs(n):
                    r0 = 8 * n
                    h_sb = []
                    for k in range(4):
                        t = sA.tile([128, 512], F32R, tag=f"h{k}", name=f"h{k}")
                        nc.sync.dma_start(
                            out=t[:],
                            in_=h_ap[128 * k:128 * (k + 1), 512 * n:512 * (n + 1)],
                        )
                        h_sb.append(t)
                    xpads = []
                    for ci in range(3):
                        pc = XCH[ci]
                        xp = sA.tile([128, 10, 66], F32R, tag=f"xpad{ci}",
                                     name=f"xpad{ci}")
                        nc.sync.dma_start(
                            out=xp[:pc, :, :],
                            in_=x_ap[128 * ci:128 * ci + pc, r0:r0 + 10, :],
                        )
                        xpads.append(xp)
                    return h_sb, xpads

                # startup-critical first: dw-x weights + slab 0/1 inputs, so the
                # first matmuls aren't queued behind the bulk weight transfers
                dwx_t = wA.tile([128, 3, 1152], F32R, tag="dwx", name="dwx")
                for _ci in range(3):
                    nc.sync.dma_start(out=dwx_t[:, _ci, :], in_=dwxd.ap()[:, _ci, :])
                early = {0: emit_slab_inputs(0)}
                wy_t = wA.tile([128, 7, 512], F32R, tag="wy", name="wy")
                for _k in range(7):
                    nc.sync.dma_start(out=wy_t[:, _k, :], in_=wyd.ap()[:, _k, :])
                wyb_t = wA.tile([128, 4], F32, tag="wyb", name="wyb")
                nc.sync.dma_start(out=wyb_t[:], in_=wybd.ap())
                early[1] = emit_slab_inputs(1)
                # prefetch phase-B weights while phase A computes
                wg_t = wB.tile([128, 16, 512], F32R, tag="wg", name="wg")
                for _k in range(16):
                    nc.sync.dma_start(out=wg_t[:, _k, :], in_=wgd.ap()[:, _k, :])
                dwi_t = wB.tile([128, 4, 1152], F32R, tag="dwi", name="dwi")
                for _ci in range(4):
                    nc.sync.dma_start(out=dwi_t[:, _ci, :], in_=dwid.ap()[:, _ci, :])

                for n in range(NCHUNK):
                    r0 = 8 * n
                    if n in early:
                        h_sb, xpads = early[n]
                    else:
                        h_sb, xpads = emit_slab_inputs(n)

                    # depthwise 3x3 on x: 9 diag matmuls per chunk -> PSUM
                    xw_sb = []
                    for ci in range(3):
                        pc = XCH[ci]
                        ps = psxw.tile([128, 8, 64], F32, tag="psxw", name="psxw")
                        for t, (dy, dx) in enumerate(taps):
                            nc.tensor.matmul(
                                ps[:pc, :, :],
                                dwx_t[:pc, ci, 128 * t:128 * t + pc],
                                xpads[ci][:pc, 1 + dy:9 + dy, 1 + dx:65 + dx],
                                start=(t == 0),
                                stop=(t == 8),
                            )
                        xw = sA.tile([128, 512], F32R, tag=f"xw{ci}",
                                     name=f"xw{ci}", bufs=1)
                        nc.scalar.copy(xw[:pc, :], ps[:pc, :, :])
                        xw_sb.append(xw)

                    # i = Wy @ [h; xw] + bias -> i_pad interior rows
                    for m in range(4):
                        ps = psi.tile([128, 512], F32, tag="psi", name="psi")
                        for k in range(4):  # h chunks first (ready earlier)
                            nc.tensor.matmul(
                                ps[:, :],
                                wy_t[:, k, 128 * m:128 * (m + 1)],
                                h_sb[k][:, :],
                                start=(k == 0),
                                stop=False,
                            )
                        for j in range(3):
                            pc = XCH[j]
                            nc.tensor.matmul(
                                ps[:, :],
                                wy_t[:pc, 4 + j, 128 * m:128 * (m + 1)],
                                xw_sb[j][:pc, :],
                                start=False,
                                stop=(j == 2),
                            )
                        nc.scalar.activation(
                            i_pad[m][:, 1 + r0:9 + r0, 1:65],
                            ps[:, :],
                            AF.Identity,
                            bias=wyb_t[:, m:m + 1],
                            scale=1.0,
                        )

            # ---------------- phase B ----------------
            with (
                tc.tile_pool(name="sB", bufs=2) as sB,
                tc.tile_pool(name="psb", bufs=2, space="PSUM") as psb,
                tc.tile_pool(name="psg", bufs=6, space="PSUM") as psg,
            ):
                for n in range(NCHUNK):
                    r0 = 8 * n
                    # depthwise 3x3 on i -> b
                    b_sb = []
                    for ci in range(4):
                        ps = psb.tile([128, 8, 64], F32, tag="psb", name="psb")
                        for t, (dy, dx) in enumerate(taps):
                            nc.tensor.matmul(
                                ps[:, :, :],
                                dwi_t[:, ci, 128 * t:128 * (t + 1)],
                                i_pad[ci][:, 1 + r0 + dy:9 + r0 + dy, 1 + dx:65 + dx],
                                start=(t == 0),
                                stop=(t == 8),
                            )
                        bt = sB.tile([128, 512], F32R, tag=f"b{ci}", name=f"b{ci}")
                        nc.scalar.copy(bt[:, :], ps[:, :, :])
                        b_sb.append(bt)

                    for m in range(4):
                        c_t = sB.tile([128, 512], F32, tag="c", name="c")
                        nc.sync.dma_start(
                            out=c_t[:],
                            in_=c_ap[128 * m:128 * (m + 1), 512 * n:512 * (n + 1)],
                        )
                        # gates: 0=i 1=f 2=c 3=o
                        sig = []
                        for g in range(4):
                            ps = psg.tile([128, 512], F32, tag="psg", name="psg")
                            for k in range(4):
                                nc.tensor.matmul(
                                    ps[:, :],
                                    wg_t[:, 4 * g + k, 128 * m:128 * (m + 1)],
                                    b_sb[k][:, :],
                                    start=(k == 0),
                                    stop=(k == 3),
                                )
                            st = sB.tile([128, 512], F32, tag=f"sg{g}", name=f"sg{g}")
                            nc.scalar.activation(
                                st[:, :], ps[:, :],
                                AF.Relu if g == 2 else AF.Sigmoid,
                            )
                            sig.append(st)

                        u1 = sB.tile([128, 512], F32, tag="u1", name="u1")
                        nc.vector.tensor_mul(u1[:, :], sig[1][:, :], c_t[:, :])
                        u2 = sB.tile([128, 512], F32, tag="u2", name="u2")
                        nc.vector.scalar_tensor_tensor(
                            out=u2[:, :], in0=sig[2][:, :], scalar=6.0,
                            in1=sig[0][:, :], op0=ALU.min, op1=ALU.mult,
                        )
                        cc_t = sB.tile([128, 512], F32, tag="cc", name="cc")
                        nc.vector.tensor_add(cc_t[:, :], u1[:, :], u2[:, :])
                        nc.sync.dma_start(
                            out=cc_ap[128 * m:128 * (m + 1), 512 * n:512 * (n + 1)],
                            in_=cc_t[:],
                        )
                        rcc = sB.tile([128, 512], F32, tag="rcc", name="rcc")
                        nc.vector.tensor_scalar(
                            out=rcc[:, :], in0=cc_t[:, :],
                            scalar1=0.0, scalar2=6.0, op0=ALU.max, op1=ALU.min,
                        )
                        ch_t = sB.tile([128, 512], F32, tag="ch", name="ch")
                        nc.vector.tensor_mul(ch_t[:, :], rcc[:, :], sig[3][:, :])
                        nc.sync.dma_start(
                            out=ch_ap[128 * m:128 * (m + 1), 512 * n:512 * (n + 1)],
                            in_=ch_t[:],
                        )

    nc.compile()
    return nc


def pack_weights(W_dw, W_dwb, Wy, Wy_b, Wi, Wbi, Wbf, Wbc, Wbo):
    WyT = Wy[:, :, 0, 0].T.astype(np.float32)  # (832, 512) lhsT
    wy = np.zeros((128, 7, 512), np.float32)
    for k in range(4):  # h chunks first
        wy[:, k, :] = WyT[320 + 128 * k:320 + 128 * (k + 1), :]
    for k in range(2):
        wy[:, 4 + k, :] = WyT[128 * k:128 * (k + 1), :]
    wy[:64, 6, :] = WyT[256:320, :]

    wyb = (Wy_b + Wy[:, :320, 0, 0] @ W_dwb).astype(np.float32)
    wyb = wyb.reshape(4, 128).T.copy()

    def diag_pack(Wtaps, nch, nchunk):
        out = np.zeros((128, nchunk, 1152), np.float32)
        w = Wtaps[:, 0].reshape(nch, 9)  # (nch, 9) tap-major (dy,dx)
        for ci in range(nchunk):
            pc = min(128, nch - 128 * ci)
            for t in range(9):
                idx = np.arange(pc)
                out[idx, ci, 128 * t + idx] = w[128 * ci + idx, t]
        return out

    dwx = diag_pack(W_dw, CIN, 3)
    dwi = diag_pack(Wi, CH, 4)

    wg = np.zeros((128, 16, 512), np.float32)
    for g, W in enumerate([Wbi, Wbf, Wbc, Wbo]):
        lhsT = W[:, :, 0, 0].T.astype(np.float32)  # (512 in, 512 out)
        for k in range(4):
            wg[:, 4 * g + k, :] = lhsT[128 * k:128 * (k + 1), :]

    return {
        "wy": wy, "wyb": np.ascontiguousarray(wyb), "wg": wg,
        "dwx": dwx, "dwi": dwi,
    }


_CACHE = {}


def _get_nc():
    if "nc" not in _CACHE:
        _CACHE["nc"] = build_nc()
    return _CACHE["nc"]


def run(inputs, trace=False, tmpdir=None):
    """inputs: dict as from setup_inputs(). Returns ((ch, cc), results_obj)."""
    inp = {k: np.asarray(v, np.float32) for k, v in inputs.items()}
    packed = pack_weights(
        inp["W_dw"], inp["W_dwb"], inp["Wy"], inp["Wy_b"], inp["Wi"],
        inp["Wbi"], inp["Wbf"], inp["Wbc"], inp["Wbo"],
    )
    xpad_host = np.zeros((B, CIN, 66, 66), np.float32)
    xpad_host[:, :, 1:65, 1:65] = inp["x"]
    in_maps = []
    for b in range(B):
        in_maps.append({
            "x": xpad_host[b],
            "h": np.ascontiguousarray(inp["h"][b].reshape(CH, PIX)),
            "c": np.ascontiguousarray(inp["c"][b].reshape(CH, PIX)),
            "zz": np.zeros((128, 128), np.float32),
            **packed,
        })
    nc = _get_nc()
    kwargs = {}
    if trace:
        _enable_trace_hooks()
        kwargs = dict(trace=True, trace_cores=[0])
        if tmpdir:
            kwargs["tmpdir"] = tmpdir
    res = run_bass_kernel_spmd(nc, in_maps, core_ids=list(range(NCORES)), **kwargs)
    ch = np.stack([res.results[b]["och"].reshape(CH, HW, HW) for b in range(B)])
    cc = np.stack([res.results[b]["occ"].reshape(CH, HW, HW) for b in range(B)])
    return (ch, cc), res


def kernel(**inputs):
    (ch, cc), _ = run(inputs, trace=False)
    return ch, cc


# ---------- optional NTFF tracing support (test harness only) ----------

def _enable_trace_hooks():
    import types, ctypes, contextlib
    if "antenv.axon_hooks" in sys.modules:
        return
    import concourse.bass_utils as bass_utils

    def _ntff_profile_via_ctypes(so_path):
        lib = ctypes.CDLL(so_path)
        if not hasattr(lib, "axon_start_nrt_profile"):
            return None
        lib.axon_start_nrt_profile.argtypes = [
            ctypes.POINTER(ctypes.c_int64), ctypes.c_size_t]
        lib.axon_start_nrt_profile.restype = ctypes.c_int64
        lib.axon_stop_nrt_profile.argtypes = [ctypes.c_char_p]
        lib.axon_stop_nrt_profile.restype = ctypes.c_int64

        @contextlib.contextmanager
        def _hook(output_dir, device_ids):
            import jax
            jax.devices()
            if device_ids:
                ids = (ctypes.c_int64 * len(device_ids))(*device_ids)
                rc = lib.axon_start_nrt_profile(ids, len(device_ids))
            else:
                rc = lib.axon_start_nrt_profile(None, 0)
            if rc != 0:
                raise RuntimeError(f"axon_start_nrt_profile rc={rc}")
            try:
                yield
            finally:
                lib.axon_stop_nrt_profile(str(output_dir).encode())
        return _hook

    hook = _ntff_profile_via_ctypes("/opt/axon/libaxon_pjrt.so")
    mod = types.ModuleType("antenv.axon_hooks")
    mod.get_axon_ntff_profile_hook = lambda: hook
    mod.set_axon_ntff_profile_hook = lambda h: None
    sys.modules["antenv.axon_hooks"] = mod
    bass_utils.upload_artifacts = lambda tmpdir: "local://" + str(tmpdir)



# revision 2
# speedup vs baseline: 1.1285x; 1.1285x over previous
"""BottleneckLSTMCell fused kernel for 8 Trainium2 NeuronCores.

Sharding: data-parallel over batch (B=8 -> 1 image per core). Each core runs
the full cell for its image:

  phase A: xw = dw3x3(x) (+bias folded into the Wy bias); i = Wy @ [h; xw] + b
  phase B: b = dw3x3(i); four 1x1 gate matmuls; LSTM pointwise -> (ch, cc)

All matmul operands are bf16 (PSUM accumulation stays fp32); pointwise math
runs on bf16 SBUF tiles (2x DVE mode). Depthwise convs run on the tensor
engine as 9 per-tap diagonal matmuls accumulating in PSUM, reading from
zero-padded SBUF-resident images so SAME padding comes free.
"""

import sys

if '/opt/trn_rl_repo' not in sys.path:
    sys.path.insert(0, '/opt/trn_rl_repo')

import numpy as np
import ml_dtypes

import concourse.bass as bass  # noqa: F401
from concourse import bacc
import concourse.mybir as mybir
from concourse.tile import TileContext
from concourse.bass_utils import run_bass_kernel_spmd

F32 = mybir.dt.float32
BF16 = mybir.dt.bfloat16
NPBF16 = ml_dtypes.bfloat16
AF = mybir.ActivationFunctionType
ALU = mybir.AluOpType

B, CIN, CH, HW = 8, 320, 512, 64
PIX = HW * HW          # 4096
NCORES = 8
NCHUNK = 8             # spatial slabs of 8 rows (512 px)
XCH = [128, 128, 64]   # x channel chunk sizes (320)


def build_nc():
    nc = bacc.Bacc(None, target_bir_lowering=False, num_devices=NCORES)

    xd = nc.dram_tensor("x", (CIN, 66, 66), BF16, kind="ExternalInput")
    hd = nc.dram_tensor("h", (CH, PIX), BF16, kind="ExternalInput")
    cd = nc.dram_tensor("c", (CH, PIX), BF16, kind="ExternalInput")
    wyd = nc.dram_tensor("wy", (128, 7, 512), BF16, kind="ExternalInput")
    wybd = nc.dram_tensor("wyb", (128, 4), F32, kind="ExternalInput")
    wgd = nc.dram_tensor("wg", (128, 16, 512), BF16, kind="ExternalInput")
    dwxd = nc.dram_tensor("dwx", (128, 3, 1152), BF16, kind="ExternalInput")
    dwid = nc.dram_tensor("dwi", (128, 4, 1152), BF16, kind="ExternalInput")
    ccd = nc.dram_tensor("occ", (CH, PIX), BF16, kind="ExternalOutput")
    chd = nc.dram_tensor("och", (CH, PIX), BF16, kind="ExternalOutput")

    x_ap, h_ap, c_ap = xd.ap(), hd.ap(), cd.ap()
    cc_ap, ch_ap = ccd.ap(), chd.ap()

    taps = [(t // 3 - 1, t % 3 - 1) for t in range(9)]

    with TileContext(nc) as tc:
        with tc.tile_pool(name="persist", bufs=1) as pp, \
             tc.tile_pool(name="wB", bufs=1) as wB:
            # i image, zero-padded on all sides: [66 rows x 66 cols]
            i_pad = [pp.tile([128, 66, 66], BF16, tag=f"ipad{m}", name=f"ipad{m}")
                     for m in range(4)]
            for m in range(4):
                nc.vector.memset(i_pad[m][:, 0, :], 0.0)
                nc.vector.memset(i_pad[m][:, 65, :], 0.0)
                nc.vector.memset(i_pad[m][:, :, 0], 0.0)
                nc.vector.memset(i_pad[m][:, :, 65], 0.0)

            # ---------------- phase A ----------------
            with (
                tc.tile_pool(name="wA", bufs=1) as wA,
                tc.tile_pool(name="sA", bufs=2) as sA,
                tc.tile_pool(name="psxw", bufs=3, space="PSUM") as psxw,
                tc.tile_pool(name="psi", bufs=4, space="PSUM") as psi,
            ):
                def emit_slab_inputs(n):
                    r0 = 8 * n
                    ht = sA.tile([128, 4, 512], BF16, tag="h", name="h")
                    nc.sync.dma_start(
                        out=ht[:],
                        in_=h_ap[:, 512 * n:512 * (n + 1)].rearrange(
                            "(k p) x -> p k x", p=128),
                    )
                    xpads = []
                    for ci in range(3):
                        pc = XCH[ci]
                        xp = sA.tile([128, 10, 66], BF16, tag=f"xpad{ci}",
                                     name=f"xpad{ci}")
                        nc.sync.dma_start(
                            out=xp[:pc, :, :],
                            in_=x_ap[128 * ci:128 * ci + pc, r0:r0 + 10, :],
                        )
                        xpads.append(xp)
                    return ht, xpads

                # startup-critical first: dw-x weights + slab 0/1 inputs, so the
                # first matmuls aren't queued behind the bulk weight transfers
                dwx_t = wA.tile([128, 3, 1152], BF16, tag="dwx", name="dwx")
                for _ci in range(3):
                    nc.sync.dma_start(out=dwx_t[:, _ci, :], in_=dwxd.ap()[:, _ci, :])
                early = {0: emit_slab_inputs(0)}
                wy_t = wA.tile([128, 7, 512], BF16, tag="wy", name="wy")
                for _k in range(7):
                    nc.sync.dma_start(out=wy_t[:, _k, :], in_=wyd.ap()[:, _k, :])
                wyb_t = wA.tile([128, 4], F32, tag="wyb", name="wyb")
                nc.sync.dma_start(out=wyb_t[:], in_=wybd.ap())
                early[1] = emit_slab_inputs(1)
                # prefetch phase-B weights while phase A computes
                wg_t = wB.tile([128, 16, 512], BF16, tag="wg", name="wg")
                for _k in range(16):
                    nc.sync.dma_start(out=wg_t[:, _k, :], in_=wgd.ap()[:, _k, :])
                dwi_t = wB.tile([128, 4, 1152], BF16, tag="dwi", name="dwi")
                for _ci in range(4):
                    nc.sync.dma_start(out=dwi_t[:, _ci, :], in_=dwid.ap()[:, _ci, :])

                for n in range(NCHUNK):
                    r0 = 8 * n
                    if n in early:
                        ht, xpads = early[n]
                    else:
                        ht, xpads = emit_slab_inputs(n)

                    # depthwise 3x3 on x: 9 diag matmuls per chunk -> PSUM
                    xw_sb = []
                    for ci in range(3):
                        pc = XCH[ci]
                        ps = psxw.tile([128, 8, 64], F32, tag="psxw", name="psxw")
                        for t, (dy, dx) in enumerate(taps):
                            nc.tensor.matmul(
                                ps[:pc, :, :],
                                dwx_t[:pc, ci, 128 * t:128 * t + pc],
                                xpads[ci][:pc, 1 + dy:9 + dy, 1 + dx:65 + dx],
                                start=(t == 0),
                                stop=(t == 8),
                            )
                        xw = sA.tile([128, 512], BF16, tag=f"xw{ci}",
                                     name=f"xw{ci}", bufs=1)
                        nc.scalar.copy(xw[:pc, :], ps[:pc, :, :])
                        xw_sb.append(xw)

                    # i = Wy @ [h; xw] + bias -> i_pad interior rows
                    for m in range(4):
                        ps = psi.tile([128, 512], F32, tag="psi", name="psi")
                        for k in range(4):  # h chunks first (ready earlier)
                            nc.tensor.matmul(
                                ps[:, :],
                                wy_t[:, k, 128 * m:128 * (m + 1)],
                                ht[:, k, :],
                                start=(k == 0),
                                stop=False,
                            )
                        for j in range(3):
                            pc = XCH[j]
                            nc.tensor.matmul(
                                ps[:, :],
                                wy_t[:pc, 4 + j, 128 * m:128 * (m + 1)],
                                xw_sb[j][:pc, :],
                                start=False,
                                stop=(j == 2),
                            )
                        nc.scalar.activation(
                            i_pad[m][:, 1 + r0:9 + r0, 1:65],
                            ps[:, :],
                            AF.Identity,
                            bias=wyb_t[:, m:m + 1],
                            scale=1.0,
                        )

            # ---------------- phase B ----------------
            with (
                tc.tile_pool(name="sB", bufs=2) as sB,
                tc.tile_pool(name="psb", bufs=2, space="PSUM") as psb,
                tc.tile_pool(name="psg", bufs=6, space="PSUM") as psg,
            ):
                for n in range(NCHUNK):
                    r0 = 8 * n
                    # depthwise 3x3 on i -> b
                    b_sb = []
                    for ci in range(4):
                        ps = psb.tile([128, 8, 64], F32, tag="psb", name="psb")
                        for t, (dy, dx) in enumerate(taps):
                            nc.tensor.matmul(
                                ps[:, :, :],
                                dwi_t[:, ci, 128 * t:128 * (t + 1)],
                                i_pad[ci][:, 1 + r0 + dy:9 + r0 + dy, 1 + dx:65 + dx],
                                start=(t == 0),
                                stop=(t == 8),
                            )
                        bt = sB.tile([128, 512], BF16, tag=f"b{ci}", name=f"b{ci}")
                        nc.scalar.copy(bt[:, :], ps[:, :, :])
                        b_sb.append(bt)

                    for m in range(4):
                        c_t = sB.tile([128, 512], BF16, tag="c", name="c")
                        nc.sync.dma_start(
                            out=c_t[:],
                            in_=c_ap[128 * m:128 * (m + 1), 512 * n:512 * (n + 1)],
                        )
                        # gates: 0=i 1=f 2=c 3=o
                        sig = []
                        for g in range(4):
                            ps = psg.tile([128, 512], F32, tag="psg", name="psg")
                            for k in range(4):
                                nc.tensor.matmul(
                                    ps[:, :],
                                    wg_t[:, 4 * g + k, 128 * m:128 * (m + 1)],
                                    b_sb[k][:, :],
                                    start=(k == 0),
                                    stop=(k == 3),
                                )
                            st = sB.tile([128, 512], BF16, tag=f"sg{g}",
                                         name=f"sg{g}")
                            if g == 2:
                                # relu6 on DVE (tensor_scalar clamp), frees ACT
                                nc.vector.tensor_scalar(
                                    out=st[:, :], in0=ps[:, :],
                                    scalar1=0.0, scalar2=6.0,
                                    op0=ALU.max, op1=ALU.min,
                                )
                            else:
                                nc.scalar.activation(st[:, :], ps[:, :],
                                                     AF.Sigmoid)
                            sig.append(st)

                        u1 = sB.tile([128, 512], BF16, tag="u1", name="u1")
                        nc.vector.tensor_mul(u1[:, :], sig[1][:, :], c_t[:, :])
                        u2 = sB.tile([128, 512], BF16, tag="u2", name="u2")
                        nc.vector.tensor_mul(u2[:, :], sig[2][:, :], sig[0][:, :])
                        cc_t = sB.tile([128, 512], BF16, tag="cc", name="cc")
                        nc.vector.tensor_add(cc_t[:, :], u1[:, :], u2[:, :])
                        nc.sync.dma_start(
                            out=cc_ap[128 * m:128 * (m + 1), 512 * n:512 * (n + 1)],
                            in_=cc_t[:],
                        )
                        rcc = sB.tile([128, 512], BF16, tag="rcc", name="rcc")
                        nc.vector.tensor_scalar(
                            out=rcc[:, :], in0=cc_t[:, :],
                            scalar1=0.0, scalar2=6.0, op0=ALU.max, op1=ALU.min,
                        )
                        ch_t = sB.tile([128, 512], BF16, tag="ch", name="ch")
                        nc.vector.tensor_mul(ch_t[:, :], rcc[:, :], sig[3][:, :])
                        nc.sync.dma_start(
                            out=ch_ap[128 * m:128 * (m + 1), 512 * n:512 * (n + 1)],
                            in_=ch_t[:],
                        )

    nc.compile()
    return nc


def pack_weights(W_dw, W_dwb, Wy, Wy_b, Wi, Wbi, Wbf, Wbc, Wbo):
    WyT = Wy[:, :, 0, 0].T.astype(np.float32)  # (832, 512) lhsT
    wy = np.zeros((128, 7, 512), np.float32)
    for k in range(4):  # h chunks first
        wy[:, k, :] = WyT[320 + 128 * k:320 + 128 * (k + 1), :]
    for k in range(2):
        wy[:, 4 + k, :] = WyT[128 * k:128 * (k + 1), :]
    wy[:64, 6, :] = WyT[256:320, :]

    wyb = (Wy_b + Wy[:, :320, 0, 0] @ W_dwb).astype(np.float32)
    wyb = wyb.reshape(4, 128).T.copy()

    def diag_pack(Wtaps, nch, nchunk):
        out = np.zeros((128, nchunk, 1152), np.float32)
        w = Wtaps[:, 0].reshape(nch, 9)  # (nch, 9) tap-major (dy,dx)
        for ci in range(nchunk):
            pc = min(128, nch - 128 * ci)
            for t in range(9):
                idx = np.arange(pc)
                out[idx, ci, 128 * t + idx] = w[128 * ci + idx, t]
        return out

    dwx = diag_pack(W_dw, CIN, 3)
    dwi = diag_pack(Wi, CH, 4)

    wg = np.zeros((128, 16, 512), np.float32)
    for g, W in enumerate([Wbi, Wbf, Wbc, Wbo]):
        lhsT = W[:, :, 0, 0].T.astype(np.float32)  # (512 in, 512 out)
        for k in range(4):
            wg[:, 4 * g + k, :] = lhsT[128 * k:128 * (k + 1), :]

    bf = lambda a: np.ascontiguousarray(a).astype(NPBF16)
    return {
        "wy": bf(wy), "wyb": np.ascontiguousarray(wyb), "wg": bf(wg),
        "dwx": bf(dwx), "dwi": bf(dwi),
    }


_CACHE = {}


def _get_nc():
    if "nc" not in _CACHE:
        _CACHE["nc"] = build_nc()
    return _CACHE["nc"]


def run(inputs, trace=False, tmpdir=None):
    """inputs: dict as from setup_inputs(). Returns ((ch, cc), results_obj)."""
    inp = {k: np.asarray(v, np.float32) for k, v in inputs.items()}
    packed = pack_weights(
        inp["W_dw"], inp["W_dwb"], inp["Wy"], inp["Wy_b"], inp["Wi"],
        inp["Wbi"], inp["Wbf"], inp["Wbc"], inp["Wbo"],
    )
    xpad_host = np.zeros((B, CIN, 66, 66), NPBF16)
    xpad_host[:, :, 1:65, 1:65] = inp["x"].astype(NPBF16)
    h_host = inp["h"].reshape(B, CH, PIX).astype(NPBF16)
    c_host = inp["c"].reshape(B, CH, PIX).astype(NPBF16)
    in_maps = []
    for b in range(B):
        in_maps.append({
            "x": xpad_host[b],
            "h": np.ascontiguousarray(h_host[b]),
            "c": np.ascontiguousarray(c_host[b]),
            **packed,
        })
    nc = _get_nc()
    kwargs = {}
    if trace:
        _enable_trace_hooks()
        kwargs = dict(trace=True, trace_cores=[0])
        if tmpdir:
            kwargs["tmpdir"] = tmpdir
    res = run_bass_kernel_spmd(nc, in_maps, core_ids=list(range(NCORES)), **kwargs)
    ch = np.stack([res.results[b]["och"].astype(np.float32).reshape(CH, HW, HW)
                   for b in range(B)])
    cc = np.stack([res.results[b]["occ"].astype(np.float32).reshape(CH, HW, HW)
                   for b in range(B)])
    return (ch, cc), res


def kernel(**inputs):
    (ch, cc), _ = run(inputs, trace=False)
    return ch, cc


# ---------- optional NTFF tracing support (test harness only) ----------

def _enable_trace_hooks():
    import types, ctypes, contextlib
    if "antenv.axon_hooks" in sys.modules:
        return
    import concourse.bass_utils as bass_utils

    def _ntff_profile_via_ctypes(so_path):
        lib = ctypes.CDLL(so_path)
        if not hasattr(lib, "axon_start_nrt_profile"):
            return None
        lib.axon_start_nrt_profile.argtypes = [
            ctypes.POINTER(ctypes.c_int64), ctypes.c_size_t]
        lib.axon_start_nrt_profile.restype = ctypes.c_int64
        lib.axon_stop_nrt_profile.argtypes = [ctypes.c_char_p]
        lib.axon_stop_nrt_profile.restype = ctypes.c_int64

        @contextlib.contextmanager
        def _hook(output_dir, device_ids):
            import jax
            jax.devices()
            if device_ids:
                ids = (ctypes.c_int64 * len(device_ids))(*device_ids)
                rc = lib.axon_start_nrt_profile(ids, len(device_ids))
            else:
                rc = lib.axon_start_nrt_profile(None, 0)
            if rc != 0:
                raise RuntimeError(f"axon_start_nrt_profile rc={rc}")
            try:
                yield
            finally:
                lib.axon_stop_nrt_profile(str(output_dir).encode())
        return _hook

    hook = _ntff_profile_via_ctypes("/opt/axon/libaxon_pjrt.so")
    mod = types.ModuleType("antenv.axon_hooks")
    mod.get_axon_ntff_profile_hook = lambda: hook
    mod.set_axon_ntff_profile_hook = lambda h: None
    sys.modules["antenv.axon_hooks"] = mod
    bass_utils.upload_artifacts = lambda tmpdir: "local://" + str(tmpdir)
